# revision 3
# baseline (speedup 1.0000x reference)
"""SimpleGCN (3-layer GCNConv + global_add_pool + linear head) on 8 Trainium2 cores.

Strategy (self-contained; shapes hardcoded for the nn_SimpleGCN problem):
 - Nodes sharded contiguously across 8 cores by dst (12500 each).
 - Per layer, per core: t' = (h @ W) * dinv[node] for the local shard (PE),
   duplicated-bf16 rows -> AllGather so every core holds t' for all nodes
   (duplication keeps gather descriptors at 512B = full DMA rate).
 - Message aggregation: edges (incl self-loops) sorted by (dst-block, src-quarter),
   dma_gather pulls t'[src] rows (int16 indices, 4 src-quarters of 25000 rows),
   one-hot S built on-chip via is_equal(dst_local, iota), PE matmul-accumulates
   per-128-dst-block segment sums in PSUM: h'[d,f] = sum_e S[e,d]*msg[e,f].
 - h' = relu(dinv*psum + bias); transposed (PE) back to feat-major for next layer.
 - Layer 3: per-block pooling matmul into a per-core local-graph window;
   head matmul gives per-core partial logits; host sums partials + head_b.
"""
import math
import numpy as np

N_NODES = 100000
N_EDGES = 1600000
D = 128
L = 3
G = 512
NC = 8
SH = N_NODES // NC            # 12500 nodes per core
NBLK = math.ceil(SH / 128)    # 98 blocks (97 full + one of 84)
BW = [128] * (NBLK - 1) + [SH - 128 * (NBLK - 1)]
NQ = 4
QROWS = N_NODES // NQ         # 25000 (< int16 max)
WCH = 14                      # chunks per gather window (1792 idx/instruction)
NIDX = WCH * 128
MSG_BUFS = 3
S_BUFS = 2
HT_COLS = NBLK * 128          # 12544 (padded node cols)
POOLW = 256                   # per-core local pooled window


def _prep(x, edge_index, batch, Ws, bs, head_w, head_b):
    x = np.asarray(x, np.float32)
    ei = np.asarray(edge_index, np.int64)
    batch = np.asarray(batch, np.int64)
    Ws = np.asarray(Ws, np.float32)
    bs = np.asarray(bs, np.float32)
    head_w = np.asarray(head_w, np.float32)

    loops = np.arange(N_NODES, dtype=np.int64)
    src = np.concatenate([ei[0], loops])
    dst = np.concatenate([ei[1], loops])
    deg = np.bincount(dst, minlength=N_NODES).astype(np.float32)
    dinv = np.where(deg > 0, 1.0 / np.sqrt(deg), 0.0).astype(np.float32)

    # ---- per-core edge bucketing by (dst block, src quarter) ----
    core = dst // SH
    per_core = []
    counts = np.zeros((NC, NBLK * NQ), np.int64)
    for c in range(NC):
        m = core == c
        s_c = src[m]
        dloc = dst[m] - c * SH
        b = dloc >> 7
        q = s_c // QROWS
        key = b * NQ + q
        order = np.argsort(key, kind="stable")
        counts[c] = np.bincount(key, minlength=NBLK * NQ)
        per_core.append((s_c[order], dloc[order], np.cumsum(counts[c]) - counts[c]))

    cmax = counts.max(axis=0).reshape(NBLK, NQ)            # max edges per (b,q)
    cchunks = -(-cmax // 128)                              # chunks per (b,q)
    # quarter chunk sequences: blocks in order
    qck_base = np.zeros((NBLK, NQ), np.int64)              # chunk offset within quarter
    CQ = np.zeros(NQ, np.int64)
    for qq in range(NQ):
        run = 0
        for b in range(NBLK):
            qck_base[b, qq] = run
            run += cchunks[b, qq]
        CQ[qq] = run
    NW = [int(-(-CQ[qq] // WCH)) for qq in range(NQ)]      # windows per quarter
    qwin_base = np.concatenate([[0], np.cumsum(NW)]).astype(np.int64)
    CTOT = int(sum(NW)) * WCH                              # total chunk slots
    NWmax = max(NW)

    # block -> list of (q, w, s) chunk refs; block ready window-group
    blk_chunks = []
    blk_ready = []
    for b in range(NBLK):
        refs = []
        wmax = 0
        for qq in range(NQ):
            for k in range(int(cchunks[b, qq])):
                ch = int(qck_base[b, qq]) + k
                refs.append((qq, ch // WCH, ch % WCH))
                wmax = max(wmax, ch // WCH)
        blk_chunks.append(refs)
        blk_ready.append(wmax)

    # ---- per-core slot data: gather indices + dst-local ----
    def quarter_chunk_col(qq, ch):   # global chunk slot column for (quarter, chunk)
        return (int(qwin_base[qq]) * WCH) + ch

    idx_cols = CTOT * 8
    ins_per_core = []
    pooled_base = np.zeros(NC, np.int64)
    ws_blk = [max(0, int(b * 128 * G / N_NODES) - 32) for b in range(NBLK)]
    for c in range(NC):
        s_c, dloc, starts = per_core[c]
        ixf = np.zeros(CTOT * 128, np.int64)               # src-local per slot (pad 0)
        dlf = np.full(CTOT * 128, -1.0, np.float32)        # dst-local per slot (pad -1)
        for b in range(NBLK):
            for qq in range(NQ):
                n = int(counts[c][b * NQ + qq])
                if n == 0 and cchunks[b, qq] == 0:
                    continue
                st = int(starts[b * NQ + qq])
                base = quarter_chunk_col(qq, int(qck_base[b, qq])) * 128
                ixf[base:base + n] = s_c[st:st + n] % QROWS
                dlf[base:base + n] = (dloc[st:st + n] % 128).astype(np.float32)
        # wrap indices: slot j of each window -> idx[p, wcol + j//16] with p%16 == j%16
        ix_win = ixf.reshape(CTOT // WCH, NIDX)            # per window
        arr = ix_win.reshape(-1, NIDX // 16, 16)           # [win, 112, 16]
        idx_sb = np.transpose(arr, (0, 2, 1)).reshape(CTOT // WCH, 16, NIDX // 16)
        idx_sb = np.concatenate([idx_sb] * 8, axis=1)      # replicate to 128 partitions
        idx_sb = np.transpose(idx_sb, (1, 0, 2)).reshape(128, idx_cols)
        dl_sb = dlf.reshape(CTOT, 128).T.copy()            # [128, CTOT]

        xT = np.zeros((128, HT_COLS), np.float32)
        xT[:, :SH] = x[c * SH:(c + 1) * SH].T
        dinv_c = np.ones((128, NBLK), np.float32)
        dv = dinv[c * SH:(c + 1) * SH]
        dinv_c.reshape(-1)[:0] = 0  # no-op, keep layout clear
        for b in range(NBLK):
            dinv_c[:BW[b], b] = dv[b * 128:b * 128 + BW[b]]
        bl = batch[c * SH:(c + 1) * SH]
        g0 = int(bl[0])
        pooled_base[c] = g0
        brel = np.full((128, NBLK), -1.0, np.float32)
        for b in range(NBLK):
            rel = (bl[b * 128:b * 128 + BW[b]] - g0 - ws_blk[b]).astype(np.int64)
            assert rel.min() >= 0 and rel.max() < 128, (c, b, rel.min(), rel.max())
            brel[:BW[b], b] = rel.astype(np.float32)
        iota = np.broadcast_to(np.arange(128, dtype=np.float32), (128, 128)).copy()
        Wk = np.ascontiguousarray(Ws.transpose(1, 0, 2).reshape(128, L * 128))
        bias_b = np.ascontiguousarray(
            np.broadcast_to(bs[:, None, :], (L, 128, 128)).transpose(1, 0, 2).reshape(128, L * 128))
        ins_per_core.append({
            "xT": xT, "Wk": Wk, "biasb": bias_b, "dinvc": dinv_c, "brel": brel,
            "iota": iota, "hw": head_w.reshape(128, 1).astype(np.float32),
            "idx": idx_sb.astype(np.int16), "dl": dl_sb.astype(np.float32),
        })
    struct = {
        "NW": NW, "NWmax": NWmax, "CTOT": CTOT, "idx_cols": idx_cols,
        "qwin_base": qwin_base, "blk_chunks": blk_chunks, "blk_ready": blk_ready,
        "ws_blk": ws_blk, "pooled_base": pooled_base,
        "head_b": float(np.asarray(head_b).reshape(-1)[0]),
    }
    return ins_per_core, struct


def _build(struct):
    import concourse.bass as bass
    import concourse.bacc as bacc
    import concourse.mybir as mybir
    import concourse.tile as tile
    from concourse.masks import make_identity

    NW = struct["NW"]
    qwin_base = struct["qwin_base"]
    blk_chunks = struct["blk_chunks"]
    blk_ready = struct["blk_ready"]
    ws_blk = struct["ws_blk"]
    idx_cols = struct["idx_cols"]
    CTOT = struct["CTOT"]
    f32 = mybir.dt.float32
    bf16 = mybir.dt.bfloat16

    nc = bacc.Bacc("TRN2", target_bir_lowering=False, debug=False,
                   num_devices=NC, num_swdge_queues=4)
    xT_d = nc.dram_tensor("xT", [128, HT_COLS], f32, kind="ExternalInput")
    Wk_d = nc.dram_tensor("Wk", [128, L * 128], f32, kind="ExternalInput")
    bias_d = nc.dram_tensor("biasb", [128, L * 128], f32, kind="ExternalInput")
    dinv_d = nc.dram_tensor("dinvc", [128, NBLK], f32, kind="ExternalInput")
    brel_d = nc.dram_tensor("brel", [128, NBLK], f32, kind="ExternalInput")
    iota_d = nc.dram_tensor("iota", [128, 128], f32, kind="ExternalInput")
    hw_d = nc.dram_tensor("hw", [128, 1], f32, kind="ExternalInput")
    idx_d = nc.dram_tensor("idx", [128, idx_cols], mybir.dt.int16, kind="ExternalInput")
    dl_d = nc.dram_tensor("dl", [128, CTOT], f32, kind="ExternalInput")
    out_d = nc.dram_tensor("out", [1, POOLW], f32, kind="ExternalOutput")

    with tile.TileContext(nc) as tc:
        with (
            tc.tile_pool(name="const", bufs=1) as cp,
            tc.tile_pool(name="hT", bufs=2) as htp,
            tc.tile_pool(name="tev", bufs=2) as tevp,
            tc.tile_pool(name="m0", bufs=MSG_BUFS) as mp0,
            tc.tile_pool(name="m1", bufs=MSG_BUFS) as mp1,
            tc.tile_pool(name="m2", bufs=MSG_BUFS) as mp2,
            tc.tile_pool(name="m3", bufs=MSG_BUFS) as mp3,
            tc.tile_pool(name="s0", bufs=S_BUFS) as sp0,
            tc.tile_pool(name="s1", bufs=S_BUFS) as sp1,
            tc.tile_pool(name="s2", bufs=S_BUFS) as sp2,
            tc.tile_pool(name="s3", bufs=S_BUFS) as sp3,
            tc.tile_pool(name="ev", bufs=3) as evp,
            tc.tile_pool(name="psA", bufs=2, space="PSUM") as psA,
            tc.tile_pool(name="psB", bufs=2, space="PSUM") as psB,
            tc.tile_pool(name="psH", bufs=1, space="PSUM") as psH,
            tc.tile_pool(name="dram", bufs=1, space="DRAM") as dp,
        ):
            mpools = [mp0, mp1, mp2, mp3]
            spools = [sp0, sp1, sp2, sp3]
            # constants
            Wk = cp.tile([128, L * 128], f32)
            nc.sync.dma_start(Wk[:], Wk_d[:])
            biasb = cp.tile([128, L * 128], f32)
            nc.sync.dma_start(biasb[:], bias_d[:])
            dinvc = cp.tile([128, NBLK], f32)
            nc.sync.dma_start(dinvc[:], dinv_d[:])
            brel = cp.tile([128, NBLK], f32)
            nc.sync.dma_start(brel[:], brel_d[:])
            iota = cp.tile([128, 128], f32)
            nc.sync.dma_start(iota[:], iota_d[:])
            hw = cp.tile([128, 1], f32)
            nc.sync.dma_start(hw[:], hw_d[:])
            idxt = cp.tile([128, idx_cols], mybir.dt.int16)
            nc.sync.dma_start(idxt[:], idx_d[:])
            dlt = cp.tile([128, CTOT], f32)
            nc.sync.dma_start(dlt[:], dl_d[:])
            ident = cp.tile([128, 128], f32)
            make_identity(nc, ident[:])
            pooledT = cp.tile([128, POOLW], f32)
            nc.vector.memset(pooledT[:], 0.0)

            hT_dram = [dp.tile([128, HT_COLS], f32, name=f"hTd{i}") for i in range(2)]
            agin = dp.tile([SH, 256], bf16)
            agout = dp.tile([N_NODES, 256], bf16)

            def iota_bcast(w):
                a = iota[:]
                return bass.AP(a.tensor, a.offset,
                               [list(a.ap[0]), [0, w], list(a.ap[1])])

            for l in range(L):
                # ---------- phase A: t' = (h @ W_l) * dinv, dup-bf16, broadcast ----------
                for hc in range(7):
                    cols = slice(hc * 1792, (hc + 1) * 1792)
                    hTt = htp.tile([128, 1792], f32, tag="hT")
                    if l == 0:
                        nc.sync.dma_start(hTt[:], xT_d[:, cols])
                    else:
                        nc.sync.dma_start(hTt[:], hT_dram[(l + 1) % 2][:, cols])
                    for bi in range(14):
                        b = hc * 14 + bi
                        w = BW[b]
                        pt = psA.tile([128, 128], f32, tag="psA")
                        nc.tensor.matmul(pt[0:w, :], lhsT=hTt[:, bi * 128:bi * 128 + w],
                                         rhs=Wk[:, l * 128:(l + 1) * 128],
                                         start=True, stop=True)
                        tev = tevp.tile([128, 256], bf16, tag="tev")
                        nc.vector.tensor_scalar_mul(tev[0:w, 0:128], pt[0:w, :],
                                                    dinvc[0:w, l * 0 + b:b + 1])
                        nc.vector.tensor_copy(tev[0:w, 128:256], tev[0:w, 0:128])
                        nc.sync.dma_start(agin[b * 128:b * 128 + w, :], tev[0:w, :])
                # broadcast t' to all cores
                nc.gpsimd.collective_compute(
                    "AllGather", mybir.AluOpType.bypass,
                    ins=[agin.opt()], outs=[agout.opt()],
                    replica_groups=[list(range(NC))],
                )
                # ---------- phase B: gather + segment-sum matmuls ----------
                mtiles = {}
                stiles = {}
                emitted = 0

                def emit_block(b):
                    w = BW[b]
                    refs = blk_chunks[b]
                    pa = psB.tile([128, 128], f32, tag="agg")
                    for i, (qq, ww, ss) in enumerate(refs):
                        nc.tensor.matmul(
                            pa[:], lhsT=stiles[(qq, ww)][:, ss, :],
                            rhs=mtiles[(qq, ww)][:, ss, 0:128],
                            start=(i == 0), stop=(i == len(refs) - 1))
                    hs = evp.tile([128, 128], f32, tag="hs")
                    nc.vector.tensor_scalar_mul(hs[0:w, :], pa[0:w, :], dinvc[0:w, b:b + 1])
                    hs2 = evp.tile([128, 128], f32, tag="hs2")
                    nc.vector.tensor_tensor(out=hs2[0:w, :], in0=hs[0:w, :],
                                            in1=biasb[0:w, l * 128:(l + 1) * 128],
                                            op=mybir.AluOpType.add)
                    hs3 = evp.tile([128, 128], f32, tag="hs3")
                    nc.scalar.activation(hs3[0:w, :], hs2[0:w, :],
                                         mybir.ActivationFunctionType.Relu)
                    if l < 2:
                        ptr = psA.tile([128, 128], f32, tag="psA")
                        nc.tensor.transpose(ptr[:], hs3[:], ident[:])
                        hTs = evp.tile([128, 128], f32, tag="hTs")
                        nc.vector.tensor_copy(hTs[:], ptr[:])
                        nc.sync.dma_start(hT_dram[l % 2][:, b * 128:(b + 1) * 128], hTs[:])
                    else:
                        spool_t = evp.tile([128, 128], f32, tag="spool")
                        nc.vector.tensor_tensor(
                            out=spool_t[:], in0=brel[:, b:b + 1].to_broadcast([128, 128]),
                            in1=iota[:], op=mybir.AluOpType.is_equal)
                        pp = psA.tile([128, 128], f32, tag="psA")
                        nc.tensor.matmul(pp[:], lhsT=hs3[:], rhs=spool_t[:],
                                         start=True, stop=True)
                        wsb = ws_blk[b]
                        nc.vector.tensor_tensor(
                            out=pooledT[:, wsb:wsb + 128], in0=pooledT[:, wsb:wsb + 128],
                            in1=pp[:], op=mybir.AluOpType.add)

                for ww in range(struct["NWmax"]):
                    for qq in range(NQ):
                        if ww >= NW[qq]:
                            continue
                        g = mpools[qq].tile([128, WCH, 256], bf16, tag=f"msg{qq}")
                        icol = (int(qwin_base[qq]) + ww) * (NIDX // 16)
                        nc.gpsimd.dma_gather(
                            out_ap=g[:],
                            in_ap=agout[qq * QROWS:(qq + 1) * QROWS, :],
                            idxs_ap=idxt[:, icol:icol + NIDX // 16],
                            num_idxs=NIDX, num_idxs_reg=NIDX, elem_size=256,
                            single_packet=False, queue_num=qq)
                        mtiles[(qq, ww)] = g
                        st = spools[qq].tile([128, WCH, 128], bf16, tag=f"S{qq}")
                        dcol = (int(qwin_base[qq]) + ww) * WCH
                        nc.vector.tensor_tensor(
                            out=st[:],
                            in0=dlt[:, dcol:dcol + WCH].to_broadcast([128, WCH, 128]),
                            in1=iota_bcast(WCH),
                            op=mybir.AluOpType.is_equal)
                        stiles[(qq, ww)] = st
                    while emitted < NBLK and blk_ready[emitted] <= ww:
                        emit_block(emitted)
                        emitted += 1
                while emitted < NBLK:
                    emit_block(emitted)
                    emitted += 1

            # ---------- head: partial logits ----------
            ph = psH.tile([128, POOLW], f32)
            nc.tensor.matmul(ph[0:1, :], lhsT=hw[:, 0:1], rhs=pooledT[:],
                             start=True, stop=True)
            outsb = cp.tile([1, POOLW], f32)
            nc.vector.tensor_copy(outsb[:], ph[0:1, :])
            nc.sync.dma_start(out_d[:], outsb[:])
    nc.compile()
    return nc


# ---------------------------------------------------------------------------
# PJRT compile-once runner (inlined; mirrors concourse.bass2jax.run_bass_via_pjrt)
# ---------------------------------------------------------------------------
class _Runner:
    def __init__(self, nc, n_cores):
        import jax
        import numpy as np
        from jax.sharding import Mesh, PartitionSpec
        from jax.experimental.shard_map import shard_map
        import concourse.mybir as mybir
        from concourse import bass2jax
        from concourse.bass2jax import _bass_exec_p, partition_id_tensor

        bass2jax.install_neuronx_cc_hook()
        self.jax = jax
        self.n_cores = n_cores
        partition_name = nc.partition_id_tensor.name if nc.partition_id_tensor else None
        in_names, out_names, out_avals, zero_outs = [], [], [], []
        for alloc in nc.m.functions[0].allocations:
            if not isinstance(alloc, mybir.MemoryLocationSet):
                continue
            name = alloc.memorylocations[0].name
            if alloc.kind == "ExternalInput":
                if name != partition_name:
                    in_names.append(name)
            elif alloc.kind == "ExternalOutput":
                out_names.append(name)
                out_avals.append(jax.core.ShapedArray(tuple(alloc.tensor_shape),
                                                      mybir.dt.np(alloc.dtype)))
                zero_outs.append(np.zeros(tuple(alloc.tensor_shape),
                                          mybir.dt.np(alloc.dtype)))
        self.in_names, self.out_names = in_names, out_names
        self.out_avals, self.zero_outs = out_avals, zero_outs
        n_params, n_outs = len(in_names), len(out_avals)
        all_in = list(in_names) + list(out_names)
        if partition_name is not None:
            all_in.append(partition_name)

        def _body(*args):
            operands = list(args)
            if partition_name is not None:
                operands.append(partition_id_tensor())
            return tuple(_bass_exec_p.bind(
                *operands, out_avals=tuple(out_avals), in_names=tuple(all_in),
                out_names=tuple(out_names), lowering_input_output_aliases=(),
                sim_require_finite=False, sim_require_nnan=False, nc=nc))

        devices = jax.devices()[:n_cores]
        self.mesh = Mesh(np.asarray(devices), ("core",))
        in_specs = (PartitionSpec("core"),) * (n_params + n_outs)
        out_specs = (PartitionSpec("core"),) * n_outs
        self.sharded = jax.jit(
            shard_map(_body, mesh=self.mesh, in_specs=in_specs,
                      out_specs=out_specs, check_rep=False),
            donate_argnums=tuple(range(n_params, n_params + n_outs)),
            keep_unused=True)

    def run(self, in_maps):
        import numpy as np
        from jax.sharding import NamedSharding, PartitionSpec
        sharding = NamedSharding(self.mesh, PartitionSpec("core"))
        concat = [self.jax.device_put(
            np.concatenate([np.asarray(in_maps[c][n]) for c in range(self.n_cores)], axis=0),
            sharding) for n in self.in_names]
        zeros = [self.jax.device_put(
            np.zeros((self.n_cores * z.shape[0], *z.shape[1:]), z.dtype), sharding)
            for z in self.zero_outs]
        outs = self.sharded(*concat, *zeros)
        self.jax.block_until_ready(outs)
        return [
            {n: np.asarray(outs[i]).reshape(self.n_cores, *self.out_avals[i].shape)[c]
             for i, n in enumerate(self.out_names)}
            for c in range(self.n_cores)
        ]


_CACHE = {}


def kernel(x, edge_index, batch, Ws, bs, head_w, head_b):
    ins_per_core, struct = _prep(x, edge_index, batch, Ws, bs, head_w, head_b)
    key = "gcn"
    if key not in _CACHE:
        nc = _build(struct)
        _CACHE[key] = _Runner(nc, NC)
    runner = _CACHE[key]
    results = runner.run(ins_per_core)
    out = np.zeros(G, np.float64)
    for c in range(NC):
        part = results[c]["out"].reshape(-1)
        g0 = int(struct["pooled_base"][c])
        w = min(POOLW, G - g0)
        out[g0:g0 + w] += part[:w]
    out += struct["head_b"]
    return out.astype(np.float32)


# revision 9
# speedup vs baseline: 1.0153x; 1.0153x over previous
"""SimpleGCN (3-layer GCNConv + global_add_pool + linear head) on 8 Trainium2 cores.

Strategy (self-contained; shapes hardcoded for the nn_SimpleGCN problem):
 - Nodes sharded contiguously across 8 cores by dst (12500 each).
 - Per layer, per core: t' = (h @ W) * dinv[node] for the local shard (PE),
   duplicated-bf16 rows -> AllGather so every core holds t' for all nodes
   (duplication keeps gather descriptors at 512B = full DMA rate).
 - Message aggregation: edges (incl self-loops) sorted by (dst-block, src-quarter),
   dma_gather pulls t'[src] rows (int16 indices, 4 src-quarters of 25000 rows),
   one-hot S built on-chip via is_equal(dst_local, iota), PE matmul-accumulates
   per-128-dst-block segment sums in PSUM: h'[d,f] = sum_e S[e,d]*msg[e,f].
 - h' = relu(dinv*psum + bias); transposed (PE) back to feat-major for next layer.
 - Layer 3: per-block pooling matmul into a per-core local-graph window;
   head matmul gives per-core partial logits; host sums partials + head_b.
"""
import math
import numpy as np

N_NODES = 100000
N_EDGES = 1600000
D = 128
L = 3
G = 512
NC = 8
SH = N_NODES // NC            # 12500 nodes per core
NBLK = math.ceil(SH / 128)    # 98 blocks (97 full + one of 84)
BW = [128] * (NBLK - 1) + [SH - 128 * (NBLK - 1)]
NQ = 4
QROWS = N_NODES // NQ         # 25000 (< int16 max)
WCH = 14                      # chunks per gather window (1792 idx/instruction)
NIDX = WCH * 128
MSG_BUFS = 3
S_BUFS = 2
HT_COLS = NBLK * 128          # 12544 (padded node cols)
POOLW = 256                   # per-core local pooled window


def _prep(x, edge_index, batch, Ws, bs, head_w, head_b):
    x = np.asarray(x, np.float32)
    ei = np.asarray(edge_index, np.int64)
    batch = np.asarray(batch, np.int64)
    Ws = np.asarray(Ws, np.float32)
    bs = np.asarray(bs, np.float32)
    head_w = np.asarray(head_w, np.float32)

    loops = np.arange(N_NODES, dtype=np.int64)
    src = np.concatenate([ei[0], loops])
    dst = np.concatenate([ei[1], loops])
    deg = np.bincount(dst, minlength=N_NODES).astype(np.float32)
    dinv = np.where(deg > 0, 1.0 / np.sqrt(deg), 0.0).astype(np.float32)

    # ---- per-core edge bucketing by (dst block, src quarter) ----
    core = dst // SH
    per_core = []
    counts = np.zeros((NC, NBLK * NQ), np.int64)
    for c in range(NC):
        m = core == c
        s_c = src[m]
        dloc = dst[m] - c * SH
        b = dloc >> 7
        q = s_c // QROWS
        key = b * NQ + q
        order = np.argsort(key, kind="stable")
        counts[c] = np.bincount(key, minlength=NBLK * NQ)
        per_core.append((s_c[order], dloc[order], np.cumsum(counts[c]) - counts[c]))

    cmax = counts.max(axis=0).reshape(NBLK, NQ)            # max edges per (b,q)
    cchunks = -(-cmax // 128)                              # chunks per (b,q)
    # quarter chunk sequences: blocks in order
    qck_base = np.zeros((NBLK, NQ), np.int64)              # chunk offset within quarter
    CQ = np.zeros(NQ, np.int64)
    for qq in range(NQ):
        run = 0
        for b in range(NBLK):
            qck_base[b, qq] = run
            run += cchunks[b, qq]
        CQ[qq] = run
    NW = [int(-(-CQ[qq] // WCH)) for qq in range(NQ)]      # windows per quarter
    qwin_base = np.concatenate([[0], np.cumsum(NW)]).astype(np.int64)
    CTOT = int(sum(NW)) * WCH                              # total chunk slots
    NWmax = max(NW)

    # block -> list of (q, w, s) chunk refs; block ready window-group
    blk_chunks = []
    blk_ready = []
    for b in range(NBLK):
        refs = []
        wmax = 0
        for qq in range(NQ):
            for k in range(int(cchunks[b, qq])):
                ch = int(qck_base[b, qq]) + k
                refs.append((qq, ch // WCH, ch % WCH))
                wmax = max(wmax, ch // WCH)
        blk_chunks.append(refs)
        blk_ready.append(wmax)

    # ---- per-core slot data: gather indices + dst-local ----
    def quarter_chunk_col(qq, ch):   # global chunk slot column for (quarter, chunk)
        return (int(qwin_base[qq]) * WCH) + ch

    idx_cols = CTOT * 8
    ins_per_core = []
    pooled_base = np.zeros(NC, np.int64)
    ws_blk = [max(0, int(b * 128 * G / N_NODES) - 32) for b in range(NBLK)]
    for c in range(NC):
        s_c, dloc, starts = per_core[c]
        ixf = np.zeros(CTOT * 128, np.int64)               # src-local per slot (pad 0)
        dlf = np.full(CTOT * 128, -1.0, np.float32)        # dst-local per slot (pad -1)
        for b in range(NBLK):
            for qq in range(NQ):
                n = int(counts[c][b * NQ + qq])
                if n == 0 and cchunks[b, qq] == 0:
                    continue
                st = int(starts[b * NQ + qq])
                base = quarter_chunk_col(qq, int(qck_base[b, qq])) * 128
                ixf[base:base + n] = s_c[st:st + n] % QROWS
                dlf[base:base + n] = (dloc[st:st + n] % 128).astype(np.float32)
        # wrap indices: slot j of each window -> idx[p, wcol + j//16] with p%16 == j%16
        ix_win = ixf.reshape(CTOT // WCH, NIDX)            # per window
        arr = ix_win.reshape(-1, NIDX // 16, 16)           # [win, 112, 16]
        idx_sb = np.transpose(arr, (0, 2, 1)).reshape(CTOT // WCH, 16, NIDX // 16)
        idx_sb = np.concatenate([idx_sb] * 8, axis=1)      # replicate to 128 partitions
        idx_sb = np.transpose(idx_sb, (1, 0, 2)).reshape(128, idx_cols)
        dl_sb = dlf.reshape(CTOT, 128).T.copy()            # [128, CTOT]

        xT = np.zeros((128, HT_COLS), np.float32)
        xT[:, :SH] = x[c * SH:(c + 1) * SH].T
        dinv_c = np.ones((128, NBLK), np.float32)
        dv = dinv[c * SH:(c + 1) * SH]
        dinv_c.reshape(-1)[:0] = 0  # no-op, keep layout clear
        for b in range(NBLK):
            dinv_c[:BW[b], b] = dv[b * 128:b * 128 + BW[b]]
        bl = batch[c * SH:(c + 1) * SH]
        g0 = int(bl[0])
        pooled_base[c] = g0
        brel = np.full((128, NBLK), -1.0, np.float32)
        for b in range(NBLK):
            rel = (bl[b * 128:b * 128 + BW[b]] - g0 - ws_blk[b]).astype(np.int64)
            assert rel.min() >= 0 and rel.max() < 128, (c, b, rel.min(), rel.max())
            brel[:BW[b], b] = rel.astype(np.float32)
        iota = np.broadcast_to(np.arange(128, dtype=np.float32), (128, 128)).copy()
        Wk = np.ascontiguousarray(Ws.transpose(1, 0, 2).reshape(128, L * 128))
        bias_b = np.ascontiguousarray(
            np.broadcast_to(bs[:, None, :], (L, 128, 128)).transpose(1, 0, 2).reshape(128, L * 128))
        ins_per_core.append({
            "xT": xT, "Wk": Wk, "biasb": bias_b, "dinvc": dinv_c, "brel": brel,
            "iota": iota, "hw": head_w.reshape(128, 1).astype(np.float32),
            "idx": idx_sb.astype(np.int16), "dl": dl_sb.astype(np.float32),
        })
    struct = {
        "NW": NW, "NWmax": NWmax, "CTOT": CTOT, "idx_cols": idx_cols,
        "qwin_base": qwin_base, "blk_chunks": blk_chunks, "blk_ready": blk_ready,
        "ws_blk": ws_blk, "pooled_base": pooled_base,
        "head_b": float(np.asarray(head_b).reshape(-1)[0]),
    }
    return ins_per_core, struct


def _build(struct):
    import os
    SKIP = set(os.environ.get("GCN_SKIP", "").split(","))
    import concourse.bass as bass
    import concourse.bacc as bacc
    import concourse.mybir as mybir
    import concourse.tile as tile
    from concourse.masks import make_identity

    NW = struct["NW"]
    qwin_base = struct["qwin_base"]
    blk_chunks = struct["blk_chunks"]
    blk_ready = struct["blk_ready"]
    ws_blk = struct["ws_blk"]
    idx_cols = struct["idx_cols"]
    CTOT = struct["CTOT"]
    f32 = mybir.dt.float32
    bf16 = mybir.dt.bfloat16

    nc = bacc.Bacc("TRN2", target_bir_lowering=False, debug=False,
                   num_devices=NC, num_swdge_queues=4)
    xT_d = nc.dram_tensor("xT", [128, HT_COLS], f32, kind="ExternalInput")
    Wk_d = nc.dram_tensor("Wk", [128, L * 128], f32, kind="ExternalInput")
    bias_d = nc.dram_tensor("biasb", [128, L * 128], f32, kind="ExternalInput")
    dinv_d = nc.dram_tensor("dinvc", [128, NBLK], f32, kind="ExternalInput")
    brel_d = nc.dram_tensor("brel", [128, NBLK], f32, kind="ExternalInput")
    iota_d = nc.dram_tensor("iota", [128, 128], f32, kind="ExternalInput")
    hw_d = nc.dram_tensor("hw", [128, 1], f32, kind="ExternalInput")
    idx_d = nc.dram_tensor("idx", [128, idx_cols], mybir.dt.int16, kind="ExternalInput")
    dl_d = nc.dram_tensor("dl", [128, CTOT], f32, kind="ExternalInput")
    out_d = nc.dram_tensor("out", [1, POOLW], f32, kind="ExternalOutput")
    tsrc_d = nc.dram_tensor("tsrc", [N_NODES, 256], bf16, kind="ExternalInput") if "extsrc" in SKIP else None

    with tile.TileContext(nc) as tc:
        with (
            tc.tile_pool(name="const", bufs=1) as cp,
            tc.tile_pool(name="hT", bufs=2) as htp,
            tc.tile_pool(name="tev", bufs=2) as tevp,
            tc.tile_pool(name="m0", bufs=MSG_BUFS) as mp0,
            tc.tile_pool(name="m1", bufs=MSG_BUFS) as mp1,
            tc.tile_pool(name="m2", bufs=MSG_BUFS) as mp2,
            tc.tile_pool(name="m3", bufs=MSG_BUFS) as mp3,
            tc.tile_pool(name="s0", bufs=S_BUFS) as sp0,
            tc.tile_pool(name="s1", bufs=S_BUFS) as sp1,
            tc.tile_pool(name="s2", bufs=S_BUFS) as sp2,
            tc.tile_pool(name="s3", bufs=S_BUFS) as sp3,
            tc.tile_pool(name="ev", bufs=3) as evp,
            tc.tile_pool(name="psA", bufs=2, space="PSUM") as psA,
            tc.tile_pool(name="psB", bufs=2, space="PSUM") as psB,
            tc.tile_pool(name="psH", bufs=1, space="PSUM") as psH,
            tc.tile_pool(name="dram", bufs=1, space="DRAM") as dp,
        ):
            mpools = [mp0, mp1, mp2, mp3]
            spools = [sp0, sp1, sp2, sp3]
            # constants
            Wk = cp.tile([128, L * 128], f32)
            nc.sync.dma_start(Wk[:], Wk_d[:])
            biasb = cp.tile([128, L * 128], f32)
            nc.sync.dma_start(biasb[:], bias_d[:])
            dinvc = cp.tile([128, NBLK], f32)
            nc.sync.dma_start(dinvc[:], dinv_d[:])
            brel = cp.tile([128, NBLK], f32)
            nc.sync.dma_start(brel[:], brel_d[:])
            iota = cp.tile([128, 128], f32)
            nc.sync.dma_start(iota[:], iota_d[:])
            hw = cp.tile([128, 1], f32)
            nc.sync.dma_start(hw[:], hw_d[:])
            idxt = cp.tile([128, idx_cols], mybir.dt.int16)
            nc.sync.dma_start(idxt[:], idx_d[:])
            dlt = cp.tile([128, CTOT], f32)
            nc.sync.dma_start(dlt[:], dl_d[:])
            ident = cp.tile([128, 128], f32)
            make_identity(nc, ident[:])
            pooledT = cp.tile([128, POOLW], f32)
            nc.vector.memset(pooledT[:], 0.0)
            cstb = cp.tile([128, 256], bf16)
            nc.vector.memset(cstb[:], 0.25)

            hT_dram = [dp.tile([128, HT_COLS], f32, name=f"hTd{i}") for i in range(2)]
            agin = dp.tile([SH, 256], bf16)
            agouts = [dp.tile([N_NODES, 256], bf16, name=f"agout{i}", addr_space=("Shared" if "noshared" not in SKIP else "Local")) for i in range(L)]

            def iota_bcast(w):
                a = iota[:]
                return bass.AP(a.tensor, a.offset,
                               [list(a.ap[0]), [0, w], list(a.ap[1])])

            for l in range(L):
                # ---------- phase A: t' = (h @ W_l) * dinv, dup-bf16, broadcast ----------
                for hc in range(7):
                    cols = slice(hc * 1792, (hc + 1) * 1792)
                    hTt = htp.tile([128, 1792], f32, tag="hT")
                    if l == 0:
                        nc.sync.dma_start(hTt[:], xT_d[:, cols])
                    else:
                        nc.sync.dma_start(hTt[:], hT_dram[(l + 1) % 2][:, cols])
                    for bi in range(14):
                        b = hc * 14 + bi
                        w = BW[b]
                        pt = psA.tile([128, 128], f32, tag="psA")
                        nc.tensor.matmul(pt[0:w, :], lhsT=hTt[:, bi * 128:bi * 128 + w],
                                         rhs=Wk[:, l * 128:(l + 1) * 128],
                                         start=True, stop=True)
                        tev = tevp.tile([128, 256], bf16, tag="tev")
                        nc.vector.tensor_scalar_mul(tev[0:w, 0:128], pt[0:w, :],
                                                    dinvc[0:w, l * 0 + b:b + 1])
                        nc.vector.tensor_copy(tev[0:w, 128:256], tev[0:w, 0:128])
                        nc.sync.dma_start(agin[b * 128:b * 128 + w, :], tev[0:w, :])
                # broadcast t' to all cores
                agout = agouts[l]
                if "ag" not in SKIP:
                    nc.gpsimd.collective_compute(
                        "AllGather", mybir.AluOpType.bypass,
                        ins=[agin.opt()], outs=[agout.opt()],
                        replica_groups=[list(range(NC))],
                    )
                # ---------- phase B: gather + segment-sum matmuls ----------
                mtiles = {}
                stiles = {}
                emitted = 0

                def emit_block(b):
                    if "evict" in SKIP:
                        return
                    w = BW[b]
                    refs = blk_chunks[b]
                    pa = psB.tile([128, 128], f32, tag="agg")
                    if "mmconst" in SKIP:
                        for i, (qq, ww, ss) in enumerate(refs):
                            nc.tensor.matmul(
                                pa[:], lhsT=cstb[:, 0:128], rhs=cstb[:, 128:256],
                                start=(i == 0), stop=(i == len(refs) - 1))
                    elif "mm" not in SKIP:
                        for i, (qq, ww, ss) in enumerate(refs):
                            nc.tensor.matmul(
                                pa[:], lhsT=stiles[(qq, ww)][:, ss, :],
                                rhs=mtiles[(qq, ww)][:, ss, 0:128],
                                start=(i == 0), stop=(i == len(refs) - 1))
                    else:
                        nc.tensor.matmul(
                            pa[:], lhsT=stiles[(qq0w := blk_chunks[b][0][0], blk_chunks[b][0][1])][:, 0, :],
                            rhs=mtiles[(blk_chunks[b][0][0], blk_chunks[b][0][1])][:, 0, 0:128],
                            start=True, stop=True)
                    hs = evp.tile([128, 128], f32, tag="hs")
                    nc.vector.tensor_scalar_mul(hs[0:w, :], pa[0:w, :], dinvc[0:w, b:b + 1])
                    hs2 = evp.tile([128, 128], f32, tag="hs2")
                    nc.vector.tensor_tensor(out=hs2[0:w, :], in0=hs[0:w, :],
                                            in1=biasb[0:w, l * 128:(l + 1) * 128],
                                            op=mybir.AluOpType.add)
                    hs3 = evp.tile([128, 128], f32, tag="hs3")
                    nc.scalar.activation(hs3[0:w, :], hs2[0:w, :],
                                         mybir.ActivationFunctionType.Relu)
                    if l < 2:
                        ptr = psA.tile([128, 128], f32, tag="psA")
                        nc.tensor.transpose(ptr[:], hs3[:], ident[:])
                        hTs = evp.tile([128, 128], f32, tag="hTs")
                        nc.vector.tensor_copy(hTs[:], ptr[:])
                        nc.sync.dma_start(hT_dram[l % 2][:, b * 128:(b + 1) * 128], hTs[:])
                    else:
                        spool_t = evp.tile([128, 128], f32, tag="spool")
                        nc.vector.tensor_tensor(
                            out=spool_t[:], in0=brel[:, b:b + 1].to_broadcast([128, 128]),
                            in1=iota[:], op=mybir.AluOpType.is_equal)
                        pp = psA.tile([128, 128], f32, tag="psA")
                        nc.tensor.matmul(pp[:], lhsT=hs3[:], rhs=spool_t[:],
                                         start=True, stop=True)
                        wsb = ws_blk[b]
                        nc.vector.tensor_tensor(
                            out=pooledT[:, wsb:wsb + 128], in0=pooledT[:, wsb:wsb + 128],
                            in1=pp[:], op=mybir.AluOpType.add)

                for ww in range(struct["NWmax"]):
                    for qq in range(NQ):
                        if ww >= NW[qq]:
                            continue
                        g = mpools[qq].tile([128, WCH, 256], bf16, tag=f"msg{qq}")
                        icol = (int(qwin_base[qq]) + ww) * (NIDX // 16)
                        if "gather" not in SKIP:
                            gsrc = tsrc_d if tsrc_d is not None else agout
                            nc.gpsimd.dma_gather(
                                out_ap=g[:],
                                in_ap=gsrc[qq * QROWS:(qq + 1) * QROWS, :],
                                idxs_ap=idxt[:, icol:icol + NIDX // 16],
                                num_idxs=NIDX, num_idxs_reg=NIDX, elem_size=256,
                                single_packet=False, queue_num=qq)
                        else:
                            nc.vector.memset(g[:, 0, 0:16], 0.0)
                        mtiles[(qq, ww)] = g
                        st = spools[qq].tile([128, WCH, 128], bf16, tag=f"S{qq}")
                        dcol = (int(qwin_base[qq]) + ww) * WCH
                        if "sbuild" not in SKIP:
                            nc.vector.tensor_tensor(
                                out=st[:],
                                in0=dlt[:, dcol:dcol + WCH].to_broadcast([128, WCH, 128]),
                                in1=iota_bcast(WCH),
                                op=mybir.AluOpType.is_equal)
                        else:
                            nc.vector.memset(st[:, 0, 0:16], 0.0)
                        stiles[(qq, ww)] = st
                    while emitted < NBLK and blk_ready[emitted] <= ww:
                        emit_block(emitted)
                        emitted += 1
                while emitted < NBLK:
                    emit_block(emitted)
                    emitted += 1

            # ---------- head: partial logits ----------
            ph = psH.tile([128, POOLW], f32)
            nc.tensor.matmul(ph[0:1, :], lhsT=hw[:, 0:1], rhs=pooledT[:],
                             start=True, stop=True)
            outsb = cp.tile([1, POOLW], f32)
            nc.vector.tensor_copy(outsb[:], ph[0:1, :])
            nc.sync.dma_start(out_d[:], outsb[:])
    nc.compile()
    return nc


# ---------------------------------------------------------------------------
# PJRT compile-once runner (inlined; mirrors concourse.bass2jax.run_bass_via_pjrt)
# ---------------------------------------------------------------------------
class _Runner:
    def __init__(self, nc, n_cores):
        import jax
        import numpy as np
        from jax.sharding import Mesh, PartitionSpec
        from jax.experimental.shard_map import shard_map
        import concourse.mybir as mybir
        from concourse import bass2jax
        from concourse.bass2jax import _bass_exec_p, partition_id_tensor

        bass2jax.install_neuronx_cc_hook()
        self.jax = jax
        self.n_cores = n_cores
        partition_name = nc.partition_id_tensor.name if nc.partition_id_tensor else None
        in_names, out_names, out_avals, zero_outs = [], [], [], []
        for alloc in nc.m.functions[0].allocations:
            if not isinstance(alloc, mybir.MemoryLocationSet):
                continue
            name = alloc.memorylocations[0].name
            if alloc.kind == "ExternalInput":
                if name != partition_name:
                    in_names.append(name)
            elif alloc.kind == "ExternalOutput":
                out_names.append(name)
                out_avals.append(jax.core.ShapedArray(tuple(alloc.tensor_shape),
                                                      mybir.dt.np(alloc.dtype)))
                zero_outs.append(np.zeros(tuple(alloc.tensor_shape),
                                          mybir.dt.np(alloc.dtype)))
        self.in_names, self.out_names = in_names, out_names
        self.out_avals, self.zero_outs = out_avals, zero_outs
        n_params, n_outs = len(in_names), len(out_avals)
        all_in = list(in_names) + list(out_names)
        if partition_name is not None:
            all_in.append(partition_name)

        def _body(*args):
            operands = list(args)
            if partition_name is not None:
                operands.append(partition_id_tensor())
            return tuple(_bass_exec_p.bind(
                *operands, out_avals=tuple(out_avals), in_names=tuple(all_in),
                out_names=tuple(out_names), lowering_input_output_aliases=(),
                sim_require_finite=False, sim_require_nnan=False, nc=nc))

        devices = jax.devices()[:n_cores]
        self.mesh = Mesh(np.asarray(devices), ("core",))
        in_specs = (PartitionSpec("core"),) * (n_params + n_outs)
        out_specs = (PartitionSpec("core"),) * n_outs
        self.sharded = jax.jit(
            shard_map(_body, mesh=self.mesh, in_specs=in_specs,
                      out_specs=out_specs, check_rep=False),
            donate_argnums=tuple(range(n_params, n_params + n_outs)),
            keep_unused=True)

    def run(self, in_maps):
        import numpy as np
        from jax.sharding import NamedSharding, PartitionSpec
        sharding = NamedSharding(self.mesh, PartitionSpec("core"))
        concat = [self.jax.device_put(
            np.concatenate([np.asarray(in_maps[c][n]) for c in range(self.n_cores)], axis=0),
            sharding) for n in self.in_names]
        zeros = [self.jax.device_put(
            np.zeros((self.n_cores * z.shape[0], *z.shape[1:]), z.dtype), sharding)
            for z in self.zero_outs]
        outs = self.sharded(*concat, *zeros)
        self.jax.block_until_ready(outs)
        return [
            {n: np.asarray(outs[i]).reshape(self.n_cores, *self.out_avals[i].shape)[c]
             for i, n in enumerate(self.out_names)}
            for c in range(self.n_cores)
        ]


_CACHE = {}


def kernel(x, edge_index, batch, Ws, bs, head_w, head_b):
    ins_per_core, struct = _prep(x, edge_index, batch, Ws, bs, head_w, head_b)
    key = "gcn"
    if key not in _CACHE:
        nc = _build(struct)
        _CACHE[key] = _Runner(nc, NC)
    runner = _CACHE[key]
    results = runner.run(ins_per_core)
    out = np.zeros(G, np.float64)
    for c in range(NC):
        part = results[c]["out"].reshape(-1)
        g0 = int(struct["pooled_base"][c])
        w = min(POOLW, G - g0)
        out[g0:g0 + w] += part[:w]
    out += struct["head_b"]
    return out.astype(np.float32)


# revision 11
# speedup vs baseline: 1.4999x; 1.4773x over previous
"""SimpleGCN (3-layer GCNConv + global_add_pool + linear head) on 8 Trainium2 cores.

Strategy (self-contained; shapes hardcoded for the nn_SimpleGCN problem):
 - Nodes sharded contiguously across 8 cores by dst (12500 each).
 - Per layer, per core: t' = (h @ W) * dinv[node] for the local shard (PE),
   duplicated-bf16 rows -> AllGather so every core holds t' for all nodes
   (duplication keeps gather descriptors at 512B = full DMA rate).
 - Message aggregation: edges (incl self-loops) sorted by (dst-block, src-quarter),
   dma_gather pulls t'[src] rows (int16 indices, 4 src-quarters of 25000 rows),
   one-hot S built on-chip via is_equal(dst_local, iota), PE matmul-accumulates
   per-128-dst-block segment sums in PSUM: h'[d,f] = sum_e S[e,d]*msg[e,f].
 - h' = relu(dinv*psum + bias); transposed (PE) back to feat-major for next layer.
 - Layer 3: per-block pooling matmul into a per-core local-graph window;
   head matmul gives per-core partial logits; host sums partials + head_b.
"""
import math
import numpy as np

N_NODES = 100000
N_EDGES = 1600000
D = 128
L = 3
G = 512
NC = 8
SH = N_NODES // NC            # 12500 nodes per core
NBLK = math.ceil(SH / 128)    # 98 blocks (97 full + one of 84)
BW = [128] * (NBLK - 1) + [SH - 128 * (NBLK - 1)]
NQ = 4
QROWS = N_NODES // NQ         # 25000 (< int16 max)
WCH = 14                      # chunks per gather window (1792 idx/instruction)
NIDX = WCH * 128
MSG_BUFS = 3
S_BUFS = 2
HT_COLS = NBLK * 128          # 12544 (padded node cols)
POOLW = 256                   # per-core local pooled window


def _prep(x, edge_index, batch, Ws, bs, head_w, head_b):
    x = np.asarray(x, np.float32)
    ei = np.asarray(edge_index, np.int64)
    batch = np.asarray(batch, np.int64)
    Ws = np.asarray(Ws, np.float32)
    bs = np.asarray(bs, np.float32)
    head_w = np.asarray(head_w, np.float32)

    loops = np.arange(N_NODES, dtype=np.int64)
    src = np.concatenate([ei[0], loops])
    dst = np.concatenate([ei[1], loops])
    deg = np.bincount(dst, minlength=N_NODES).astype(np.float32)
    dinv = np.where(deg > 0, 1.0 / np.sqrt(deg), 0.0).astype(np.float32)

    # ---- per-core edge bucketing by (dst block, src quarter) ----
    core = dst // SH
    per_core = []
    counts = np.zeros((NC, NBLK * NQ), np.int64)
    for c in range(NC):
        m = core == c
        s_c = src[m]
        dloc = dst[m] - c * SH
        b = dloc >> 7
        q = s_c // QROWS
        key = b * NQ + q
        order = np.argsort(key, kind="stable")
        counts[c] = np.bincount(key, minlength=NBLK * NQ)
        per_core.append((s_c[order], dloc[order], np.cumsum(counts[c]) - counts[c]))

    cmax = counts.max(axis=0).reshape(NBLK, NQ)            # max edges per (b,q)
    cchunks = -(-cmax // 128)                              # chunks per (b,q)
    # quarter chunk sequences: blocks in order
    qck_base = np.zeros((NBLK, NQ), np.int64)              # chunk offset within quarter
    CQ = np.zeros(NQ, np.int64)
    for qq in range(NQ):
        run = 0
        for b in range(NBLK):
            qck_base[b, qq] = run
            run += cchunks[b, qq]
        CQ[qq] = run
    NW = [int(-(-CQ[qq] // WCH)) for qq in range(NQ)]      # windows per quarter
    qwin_base = np.concatenate([[0], np.cumsum(NW)]).astype(np.int64)
    CTOT = int(sum(NW)) * WCH                              # total chunk slots
    NWmax = max(NW)

    # block -> list of (q, w, s) chunk refs; block ready window-group
    blk_chunks = []
    blk_ready = []
    for b in range(NBLK):
        refs = []
        wmax = 0
        for qq in range(NQ):
            for k in range(int(cchunks[b, qq])):
                ch = int(qck_base[b, qq]) + k
                refs.append((qq, ch // WCH, ch % WCH))
                wmax = max(wmax, ch // WCH)
        blk_chunks.append(refs)
        blk_ready.append(wmax)

    # ---- per-core slot data: gather indices + dst-local ----
    def quarter_chunk_col(qq, ch):   # global chunk slot column for (quarter, chunk)
        return (int(qwin_base[qq]) * WCH) + ch

    idx_cols = CTOT * 8
    ins_per_core = []
    pooled_base = np.zeros(NC, np.int64)
    ws_blk = [max(0, int(b * 128 * G / N_NODES) - 32) for b in range(NBLK)]
    for c in range(NC):
        s_c, dloc, starts = per_core[c]
        ixf = np.zeros(CTOT * 128, np.int64)               # src-local per slot (pad 0)
        dlf = np.full(CTOT * 128, -1.0, np.float32)        # dst-local per slot (pad -1)
        for b in range(NBLK):
            for qq in range(NQ):
                n = int(counts[c][b * NQ + qq])
                if n == 0 and cchunks[b, qq] == 0:
                    continue
                st = int(starts[b * NQ + qq])
                base = quarter_chunk_col(qq, int(qck_base[b, qq])) * 128
                ixf[base:base + n] = s_c[st:st + n] % QROWS
                dlf[base:base + n] = (dloc[st:st + n] % 128).astype(np.float32)
        # wrap indices: slot j of each window -> idx[p, wcol + j//16] with p%16 == j%16
        ix_win = ixf.reshape(CTOT // WCH, NIDX)            # per window
        arr = ix_win.reshape(-1, NIDX // 16, 16)           # [win, 112, 16]
        idx_sb = np.transpose(arr, (0, 2, 1)).reshape(CTOT // WCH, 16, NIDX // 16)
        idx_sb = np.concatenate([idx_sb] * 8, axis=1)      # replicate to 128 partitions
        idx_sb = np.transpose(idx_sb, (1, 0, 2)).reshape(128, idx_cols)
        dl_sb = dlf.reshape(CTOT, 128).T.copy()            # [128, CTOT]

        xT = np.zeros((128, HT_COLS), np.float32)
        xT[:, :SH] = x[c * SH:(c + 1) * SH].T
        dinv_c = np.ones((128, NBLK), np.float32)
        dv = dinv[c * SH:(c + 1) * SH]
        dinv_c.reshape(-1)[:0] = 0  # no-op, keep layout clear
        for b in range(NBLK):
            dinv_c[:BW[b], b] = dv[b * 128:b * 128 + BW[b]]
        bl = batch[c * SH:(c + 1) * SH]
        g0 = int(bl[0])
        pooled_base[c] = g0
        brel = np.full((128, NBLK), -1.0, np.float32)
        for b in range(NBLK):
            rel = (bl[b * 128:b * 128 + BW[b]] - g0 - ws_blk[b]).astype(np.int64)
            assert rel.min() >= 0 and rel.max() < 128, (c, b, rel.min(), rel.max())
            brel[:BW[b], b] = rel.astype(np.float32)
        iota = np.broadcast_to(np.arange(128, dtype=np.float32), (128, 128)).copy()
        iota3 = np.tile(np.arange(128, dtype=np.float32), (128, WCH)).copy()
        Wk = np.ascontiguousarray(Ws.transpose(1, 0, 2).reshape(128, L * 128))
        bias_b = np.ascontiguousarray(
            np.broadcast_to(bs[:, None, :], (L, 128, 128)).transpose(1, 0, 2).reshape(128, L * 128))
        ins_per_core.append({
            "xT": xT, "Wk": Wk, "biasb": bias_b, "dinvc": dinv_c, "brel": brel,
            "iota": iota, "iota3": iota3, "hw": head_w.reshape(128, 1).astype(np.float32),
            "idx": idx_sb.astype(np.int16), "dl": dl_sb.astype(np.float32),
        })
    struct = {
        "NW": NW, "NWmax": NWmax, "CTOT": CTOT, "idx_cols": idx_cols,
        "qwin_base": qwin_base, "blk_chunks": blk_chunks, "blk_ready": blk_ready,
        "ws_blk": ws_blk, "pooled_base": pooled_base,
        "head_b": float(np.asarray(head_b).reshape(-1)[0]),
    }
    return ins_per_core, struct


def _build(struct):
    import os
    SKIP = set(os.environ.get("GCN_SKIP", "").split(","))
    import concourse.bass as bass
    import concourse.bacc as bacc
    import concourse.mybir as mybir
    import concourse.tile as tile
    from concourse.masks import make_identity

    NW = struct["NW"]
    qwin_base = struct["qwin_base"]
    blk_chunks = struct["blk_chunks"]
    blk_ready = struct["blk_ready"]
    ws_blk = struct["ws_blk"]
    idx_cols = struct["idx_cols"]
    CTOT = struct["CTOT"]
    f32 = mybir.dt.float32
    bf16 = mybir.dt.bfloat16

    nc = bacc.Bacc("TRN2", target_bir_lowering=False, debug=False,
                   num_devices=NC, num_swdge_queues=4)
    xT_d = nc.dram_tensor("xT", [128, HT_COLS], f32, kind="ExternalInput")
    Wk_d = nc.dram_tensor("Wk", [128, L * 128], f32, kind="ExternalInput")
    bias_d = nc.dram_tensor("biasb", [128, L * 128], f32, kind="ExternalInput")
    dinv_d = nc.dram_tensor("dinvc", [128, NBLK], f32, kind="ExternalInput")
    brel_d = nc.dram_tensor("brel", [128, NBLK], f32, kind="ExternalInput")
    iota_d = nc.dram_tensor("iota", [128, 128], f32, kind="ExternalInput")
    iota3_d = nc.dram_tensor("iota3", [128, WCH * 128], f32, kind="ExternalInput")
    hw_d = nc.dram_tensor("hw", [128, 1], f32, kind="ExternalInput")
    idx_d = nc.dram_tensor("idx", [128, idx_cols], mybir.dt.int16, kind="ExternalInput")
    dl_d = nc.dram_tensor("dl", [128, CTOT], f32, kind="ExternalInput")
    out_d = nc.dram_tensor("out", [1, POOLW], f32, kind="ExternalOutput")
    tsrc_d = nc.dram_tensor("tsrc", [N_NODES, 256], bf16, kind="ExternalInput") if "extsrc" in SKIP else None

    with tile.TileContext(nc) as tc:
        with (
            tc.tile_pool(name="const", bufs=1) as cp,
            tc.tile_pool(name="hT", bufs=2) as htp,
            tc.tile_pool(name="tev", bufs=2) as tevp,
            tc.tile_pool(name="m0", bufs=MSG_BUFS) as mp0,
            tc.tile_pool(name="m1", bufs=MSG_BUFS) as mp1,
            tc.tile_pool(name="m2", bufs=MSG_BUFS) as mp2,
            tc.tile_pool(name="m3", bufs=MSG_BUFS) as mp3,
            tc.tile_pool(name="s0", bufs=S_BUFS) as sp0,
            tc.tile_pool(name="s1", bufs=S_BUFS) as sp1,
            tc.tile_pool(name="s2", bufs=S_BUFS) as sp2,
            tc.tile_pool(name="s3", bufs=S_BUFS) as sp3,
            tc.tile_pool(name="ev", bufs=3) as evp,
            tc.tile_pool(name="psA", bufs=2, space="PSUM") as psA,
            tc.tile_pool(name="psB", bufs=2, space="PSUM") as psB,
            tc.tile_pool(name="psH", bufs=1, space="PSUM") as psH,
            tc.tile_pool(name="dram", bufs=1, space="DRAM") as dp,
        ):
            mpools = [mp0, mp1, mp2, mp3]
            spools = [sp0, sp1, sp2, sp3]
            # constants
            Wk = cp.tile([128, L * 128], f32)
            nc.sync.dma_start(Wk[:], Wk_d[:])
            biasb = cp.tile([128, L * 128], f32)
            nc.sync.dma_start(biasb[:], bias_d[:])
            dinvc = cp.tile([128, NBLK], f32)
            nc.sync.dma_start(dinvc[:], dinv_d[:])
            brel = cp.tile([128, NBLK], f32)
            nc.sync.dma_start(brel[:], brel_d[:])
            iota = cp.tile([128, 128], f32)
            nc.sync.dma_start(iota[:], iota_d[:])
            iota3 = cp.tile([128, WCH * 128], f32)
            nc.sync.dma_start(iota3[:], iota3_d[:])
            hw = cp.tile([128, 1], f32)
            nc.sync.dma_start(hw[:], hw_d[:])
            idxt = cp.tile([128, idx_cols], mybir.dt.int16)
            nc.sync.dma_start(idxt[:], idx_d[:])
            dlt = cp.tile([128, CTOT], f32)
            nc.sync.dma_start(dlt[:], dl_d[:])
            ident = cp.tile([128, 128], f32)
            make_identity(nc, ident[:])
            pooledT = cp.tile([128, POOLW], f32)
            nc.vector.memset(pooledT[:], 0.0)
            cstb = cp.tile([128, 256], bf16)
            nc.vector.memset(cstb[:], 0.25)

            hT_dram = [dp.tile([128, HT_COLS], f32, name=f"hTd{i}") for i in range(2)]
            agin = dp.tile([SH, 256], bf16)
            agouts = [dp.tile([N_NODES, 256], bf16, name=f"agout{i}", addr_space=("Shared" if "noshared" not in SKIP else "Local")) for i in range(L)]

            def iota_bcast(w):
                a = iota[:]
                return bass.AP(a.tensor, a.offset,
                               [list(a.ap[0]), [0, w], list(a.ap[1])])

            for l in range(L):
                # ---------- phase A: t' = (h @ W_l) * dinv, dup-bf16, broadcast ----------
                for hc in range(7):
                    cols = slice(hc * 1792, (hc + 1) * 1792)
                    hTt = htp.tile([128, 1792], f32, tag="hT")
                    if l == 0:
                        nc.sync.dma_start(hTt[:], xT_d[:, cols])
                    else:
                        nc.sync.dma_start(hTt[:], hT_dram[(l + 1) % 2][:, cols])
                    for bi in range(14):
                        b = hc * 14 + bi
                        w = BW[b]
                        pt = psA.tile([128, 128], f32, tag="psA")
                        nc.tensor.matmul(pt[0:w, :], lhsT=hTt[:, bi * 128:bi * 128 + w],
                                         rhs=Wk[:, l * 128:(l + 1) * 128],
                                         start=True, stop=True)
                        tev = tevp.tile([128, 256], bf16, tag="tev")
                        nc.vector.tensor_scalar_mul(tev[0:w, 0:128], pt[0:w, :],
                                                    dinvc[0:w, l * 0 + b:b + 1])
                        nc.vector.tensor_copy(tev[0:w, 128:256], tev[0:w, 0:128])
                        nc.sync.dma_start(agin[b * 128:b * 128 + w, :], tev[0:w, :])
                # broadcast t' to all cores
                agout = agouts[l]
                if "ag" not in SKIP:
                    nc.gpsimd.collective_compute(
                        "AllGather", mybir.AluOpType.bypass,
                        ins=[agin.opt()], outs=[agout.opt()],
                        replica_groups=[list(range(NC))],
                    )
                # ---------- phase B: gather + segment-sum matmuls ----------
                mtiles = {}
                stiles = {}
                emitted = 0

                def emit_block(b):
                    if "evict" in SKIP:
                        return
                    w = BW[b]
                    refs = blk_chunks[b]
                    pa = psB.tile([128, 128], f32, tag="agg")
                    if "mmconst" in SKIP:
                        for i, (qq, ww, ss) in enumerate(refs):
                            nc.tensor.matmul(
                                pa[:], lhsT=cstb[:, 0:128], rhs=cstb[:, 128:256],
                                start=(i == 0), stop=(i == len(refs) - 1))
                    elif "mm" not in SKIP:
                        for i, (qq, ww, ss) in enumerate(refs):
                            nc.tensor.matmul(
                                pa[:], lhsT=stiles[(qq, ww)][:, ss, :],
                                rhs=mtiles[(qq, ww)][:, ss, 0:128],
                                start=(i == 0), stop=(i == len(refs) - 1))
                    else:
                        nc.tensor.matmul(
                            pa[:], lhsT=stiles[(qq0w := blk_chunks[b][0][0], blk_chunks[b][0][1])][:, 0, :],
                            rhs=mtiles[(blk_chunks[b][0][0], blk_chunks[b][0][1])][:, 0, 0:128],
                            start=True, stop=True)
                    hs = evp.tile([128, 128], f32, tag="hs")
                    nc.vector.tensor_scalar_mul(hs[0:w, :], pa[0:w, :], dinvc[0:w, b:b + 1])
                    hs2 = evp.tile([128, 128], f32, tag="hs2")
                    nc.vector.tensor_tensor(out=hs2[0:w, :], in0=hs[0:w, :],
                                            in1=biasb[0:w, l * 128:(l + 1) * 128],
                                            op=mybir.AluOpType.add)
                    hs3 = evp.tile([128, 128], f32, tag="hs3")
                    nc.scalar.activation(hs3[0:w, :], hs2[0:w, :],
                                         mybir.ActivationFunctionType.Relu)
                    if l < 2:
                        ptr = psA.tile([128, 128], f32, tag="psA")
                        nc.tensor.transpose(ptr[:], hs3[:], ident[:])
                        hTs = evp.tile([128, 128], f32, tag="hTs")
                        nc.vector.tensor_copy(hTs[:], ptr[:])
                        nc.sync.dma_start(hT_dram[l % 2][:, b * 128:(b + 1) * 128], hTs[:])
                    else:
                        spool_t = evp.tile([128, 128], f32, tag="spool")
                        nc.vector.tensor_tensor(
                            out=spool_t[:], in0=brel[:, b:b + 1].to_broadcast([128, 128]),
                            in1=iota[:], op=mybir.AluOpType.is_equal)
                        pp = psA.tile([128, 128], f32, tag="psA")
                        nc.tensor.matmul(pp[:], lhsT=hs3[:], rhs=spool_t[:],
                                         start=True, stop=True)
                        wsb = ws_blk[b]
                        nc.vector.tensor_tensor(
                            out=pooledT[:, wsb:wsb + 128], in0=pooledT[:, wsb:wsb + 128],
                            in1=pp[:], op=mybir.AluOpType.add)

                for ww in range(struct["NWmax"]):
                    for qq in range(NQ):
                        if ww >= NW[qq]:
                            continue
                        g = mpools[qq].tile([128, WCH, 256], bf16, tag=f"msg{qq}")
                        icol = (int(qwin_base[qq]) + ww) * (NIDX // 16)
                        if "gather" not in SKIP:
                            gsrc = tsrc_d if tsrc_d is not None else agout
                            nc.gpsimd.dma_gather(
                                out_ap=g[:],
                                in_ap=gsrc[qq * QROWS:(qq + 1) * QROWS, :],
                                idxs_ap=idxt[:, icol:icol + NIDX // 16],
                                num_idxs=NIDX, num_idxs_reg=NIDX, elem_size=256,
                                single_packet=False, queue_num=qq)
                        else:
                            nc.vector.memset(g[:, 0, 0:16], 0.0)
                        mtiles[(qq, ww)] = g
                        st = spools[qq].tile([128, WCH, 128], bf16, tag=f"S{qq}")
                        dcol = (int(qwin_base[qq]) + ww) * WCH
                        if "sbuild" not in SKIP:
                            nc.vector.tensor_tensor(
                                out=st[:],
                                in0=dlt[:, dcol:dcol + WCH].to_broadcast([128, WCH, 128]),
                                in1=iota3[:].rearrange("p (w d) -> p w d", w=WCH),
                                op=mybir.AluOpType.is_equal)
                        else:
                            nc.vector.memset(st[:, 0, 0:16], 0.0)
                        stiles[(qq, ww)] = st
                    while emitted < NBLK and blk_ready[emitted] <= ww:
                        emit_block(emitted)
                        emitted += 1
                while emitted < NBLK:
                    emit_block(emitted)
                    emitted += 1

            # ---------- head: partial logits ----------
            ph = psH.tile([128, POOLW], f32)
            nc.tensor.matmul(ph[0:1, :], lhsT=hw[:, 0:1], rhs=pooledT[:],
                             start=True, stop=True)
            outsb = cp.tile([1, POOLW], f32)
            nc.vector.tensor_copy(outsb[:], ph[0:1, :])
            nc.sync.dma_start(out_d[:], outsb[:])
    nc.compile()
    return nc


# ---------------------------------------------------------------------------
# PJRT compile-once runner (inlined; mirrors concourse.bass2jax.run_bass_via_pjrt)
# ---------------------------------------------------------------------------
class _Runner:
    def __init__(self, nc, n_cores):
        import jax
        import numpy as np
        from jax.sharding import Mesh, PartitionSpec
        from jax.experimental.shard_map import shard_map
        import concourse.mybir as mybir
        from concourse import bass2jax
        from concourse.bass2jax import _bass_exec_p, partition_id_tensor

        bass2jax.install_neuronx_cc_hook()
        self.jax = jax
        self.n_cores = n_cores
        partition_name = nc.partition_id_tensor.name if nc.partition_id_tensor else None
        in_names, out_names, out_avals, zero_outs = [], [], [], []
        for alloc in nc.m.functions[0].allocations:
            if not isinstance(alloc, mybir.MemoryLocationSet):
                continue
            name = alloc.memorylocations[0].name
            if alloc.kind == "ExternalInput":
                if name != partition_name:
                    in_names.append(name)
            elif alloc.kind == "ExternalOutput":
                out_names.append(name)
                out_avals.append(jax.core.ShapedArray(tuple(alloc.tensor_shape),
                                                      mybir.dt.np(alloc.dtype)))
                zero_outs.append(np.zeros(tuple(alloc.tensor_shape),
                                          mybir.dt.np(alloc.dtype)))
        self.in_names, self.out_names = in_names, out_names
        self.out_avals, self.zero_outs = out_avals, zero_outs
        n_params, n_outs = len(in_names), len(out_avals)
        all_in = list(in_names) + list(out_names)
        if partition_name is not None:
            all_in.append(partition_name)

        def _body(*args):
            operands = list(args)
            if partition_name is not None:
                operands.append(partition_id_tensor())
            return tuple(_bass_exec_p.bind(
                *operands, out_avals=tuple(out_avals), in_names=tuple(all_in),
                out_names=tuple(out_names), lowering_input_output_aliases=(),
                sim_require_finite=False, sim_require_nnan=False, nc=nc))

        devices = jax.devices()[:n_cores]
        self.mesh = Mesh(np.asarray(devices), ("core",))
        in_specs = (PartitionSpec("core"),) * (n_params + n_outs)
        out_specs = (PartitionSpec("core"),) * n_outs
        self.sharded = jax.jit(
            shard_map(_body, mesh=self.mesh, in_specs=in_specs,
                      out_specs=out_specs, check_rep=False),
            donate_argnums=tuple(range(n_params, n_params + n_outs)),
            keep_unused=True)

    def run(self, in_maps):
        import numpy as np
        from jax.sharding import NamedSharding, PartitionSpec
        sharding = NamedSharding(self.mesh, PartitionSpec("core"))
        concat = [self.jax.device_put(
            np.concatenate([np.asarray(in_maps[c][n]) for c in range(self.n_cores)], axis=0),
            sharding) for n in self.in_names]
        zeros = [self.jax.device_put(
            np.zeros((self.n_cores * z.shape[0], *z.shape[1:]), z.dtype), sharding)
            for z in self.zero_outs]
        outs = self.sharded(*concat, *zeros)
        self.jax.block_until_ready(outs)
        return [
            {n: np.asarray(outs[i]).reshape(self.n_cores, *self.out_avals[i].shape)[c]
             for i, n in enumerate(self.out_names)}
            for c in range(self.n_cores)
        ]


_CACHE = {}


def kernel(x, edge_index, batch, Ws, bs, head_w, head_b):
    import hashlib
    ins_per_core, struct = _prep(x, edge_index, batch, Ws, bs, head_w, head_b)
    h = hashlib.sha1()
    h.update(np.ascontiguousarray(edge_index).tobytes())
    h.update(np.ascontiguousarray(batch).tobytes())
    key = h.hexdigest()
    if key not in _CACHE:
        nc = _build(struct)
        _CACHE[key] = _Runner(nc, NC)
        _CACHE["gcn"] = _CACHE[key]
    runner = _CACHE[key]
    results = runner.run(ins_per_core)
    out = np.zeros(G, np.float64)
    for c in range(NC):
        part = results[c]["out"].reshape(-1)
        g0 = int(struct["pooled_base"][c])
        w = min(POOLW, G - g0)
        out[g0:g0 + w] += part[:w]
    out += struct["head_b"]
    return out.astype(np.float32)
